# revision 21
# baseline (speedup 1.0000x reference)
"""BitLinear (1.58-bit) kernel for Trainium2, 8-core data-parallel SPMD.

Reference op: out = sign(x) @ ternarize(W).T where
  ternarize(W) = sign(W) * min(round(|W| / gamma), 1), gamma = mean(|W|) + 1e-6.

Strategy (per sharding hint: data-parallel over batch*seq, replicate ternary W):
  - Host: ternarize W once, transpose to [in, out], pack as fp8e4 bytes
    (exact for -1/0/+1).  Shard x by rows across 8 cores; send only the SIGN
    BITS of each x shard (8 contraction-slots per byte) - 0.5 MB per core
    instead of 4.2 MB, so input DMA never starves the PE.
  - Device (per core): expand sign bits to fp8 {+1,-1} bytes on DVE
    (shift/and then or-in the fp8 exponent bits), then dense fp8 DoubleRow
    matmuls (2 MACs/cell/cyc) accumulating in PSUM f32.  Products are +-1 and
    row sums <= 2048 so fp32 accumulation is exact.
  - Host: concatenate + re-tile the 8 per-core outputs.

Schedule: the 16x4 (m-tile x o-quarter) unit grid runs q-outer within
mi-blocks of 8, so each 1 MB weight quarter is first needed ~14 us after the
previous one (vs. all 4.2 MB inside the first 7 us for mi-major order).  Each
unit accumulates all 8 contraction chunks into one PSUM bank, copies to SBUF
f16 (alternating DVE/ACT), and stores its own contiguous 128 KB DRAM block -
spreading output traffic evenly and shrinking the kernel tail.

Layout: contraction index i in [0, 2048) is split as i = kc*256 + j*128 + p
(kc = 256-wide chunk, j = DoubleRow pair slot, p = SBUF partition).  Both
matmul operands are stored [128, ..., 2, N] in SBUF and sliced to the 3D
[128 part, 2, N] APs that MatmulPerfMode.DoubleRow requires.
"""

import numpy as np
import ml_dtypes

import concourse.bass as bass
import concourse.bacc as bacc
import concourse.mybir as mybir
from concourse.tile import TileContext
from concourse.bass_utils import run_bass_kernel_spmd

FP8 = ml_dtypes.float8_e4m3  # maps to mybir.dt.float8e4

N_CORES = 8
EPS = 1e-6

# Full-problem shapes (hardcoded per harness contract).
B, S, I_DIM, O_DIM = 4, 4096, 2048, 2048
M_TOT = B * S                 # 16384 rows
M_PER = M_TOT // N_CORES      # 2048 rows per core

KC = I_DIM // 256             # 8 contraction chunks
MT = M_PER // 128             # 16 output row tiles
QT = O_DIM // 512             # 4 output col quarters (one PSUM bank each)
MI_BLK = 2                    # m-tiles per schedule block (q-outer inside)

# x sign-bit DMA groups (mi ranges), weight DMA groups (kc-range, quarter),
# and x-expansion groups (mi ranges), all deadline-ordered.  Weight groups are
# spread over three HWDGE queues so their ~1.5 us per-DMA receipt latencies
# overlap instead of serializing ahead of the first matmul.
X_GROUPS = [(0, 2), (2, 8), (8, MT)]
W_GROUPS = [(0, 2, 0), (2, 4, 0), (4, 6, 0), (6, 8, 0),
            (0, 8, 1), (0, 8, 2), (0, 8, 3)]
E_GROUPS = [(0, 1), (1, 2), (2, 4), (4, 8), (8, 12), (12, MT)]


def build_program() -> bass.Bass:
    """Per-core SPMD program: out[m, o] = sign(x)[m, :] @ Wq[o, :].T.

    DRAM inputs (flat u8, concatenated per-DMA-group partition-major blocks):
      xr : pre-expanded fp8 sign bytes for mi 0..MI_RAW-1, [128p, mi, kc, 2, m]
      xp : sign bits of x^T for mi >= MI_RAW, byte [p, mi, j, m] holds bits
           kc=0..7 (bit kc = 1 iff x < 0), i = kc*256 + j*128 + p
      wt : ternary Wq^T as fp8e4 bytes, blocks [128p, kcr, 2, 512]
    DRAM output:
      out: [MT*QT*128, 512] f16; block (mi*QT + q) holds rows mi*128..+128,
           cols q*512..+512 (host re-tiles; integer values <= 2048, exact)
    """
    nc = bacc.Bacc()

    xp_total = 128 * MT * 2 * 128
    w_total = KC * 128 * 2 * O_DIM
    xp = nc.declare_dram_parameter(
        "xp", [xp_total], mybir.dt.uint8, isOutput=False)
    wt = nc.declare_dram_parameter(
        "wt", [w_total], mybir.dt.uint8, isOutput=False)
    out = nc.declare_dram_parameter(
        "out", [MT * QT * 128, 512], mybir.dt.float16, isOutput=True)

    with TileContext(nc) as tc:
        with (
            tc.tile_pool(name="wq", bufs=1) as wq_pool,
            tc.tile_pool(name="xs", bufs=1) as xs_pool,
            tc.tile_pool(name="xpk", bufs=1) as xp_pool,
            tc.tile_pool(name="psum", bufs=8, space="PSUM") as psum_pool,
            tc.tile_pool(name="osb", bufs=8) as out_pool,
        ):
            xp_sb = xp_pool.tile([128, MT, 2, 128], mybir.dt.uint8)
            xs_sb = xs_pool.tile([128, MT, KC, 2, 128], mybir.dt.float8e4)
            wq_sb = wq_pool.tile([128, KC, 2, O_DIM], mybir.dt.float8e4)

            # Every DMA group is a contiguous flat slice (per-group
            # partition-major host packing).  The wire's first ~4 us are
            # reserved for the critical set (w q0 + xp head): the bulk weight
            # quarters q1-q3 sit behind a gate read of the kc6-7 slice so
            # they cannot flood HBM while the stream-start bytes land.
            #   SP queue:   xp mi0-1 | xp mi2-7 | w (q0,kc4-5) | 64 stores
            #   ACT queue:  w (q0,kc0-1) | (q0,kc2-3) | xp mi8-15 | 64 copies
            #   POOL queue: memsets | w (q0,kc6-7) | gate | (q1) | (q2) | (q3)
            x_off = {}
            off = 0
            for b0, b1 in X_GROUPS:
                x_off[(b0, b1)] = off
                off += 128 * (b1 - b0) * 2 * 128
            w_off = {}
            off = 0
            for g in W_GROUPS:
                w_off[g] = off
                off += 128 * (g[1] - g[0]) * 2 * 512

            # Warmup scratch memsets go first on the POOL queue so the
            # blocking weight-DMA issues behind them cannot delay warmup.
            wu_a = wq_pool.tile([128, 2, 128], mybir.dt.float8e4)
            wu_b = wq_pool.tile([128, 2, 512], mybir.dt.float8e4)
            nc.gpsimd.memset(wu_a, 0.0)
            nc.gpsimd.memset(wu_b, 0.0)

            def dma_w(eng, g):
                k0, k1, q = g
                sz = 128 * (k1 - k0) * 2 * 512
                o0 = w_off[g]
                eng.dma_start(
                    out=wq_sb[:, k0:k1, :, q * 512:(q + 1) * 512].bitcast(
                        mybir.dt.uint8),
                    in_=wt[o0:o0 + sz].rearrange("(p r) -> p r", p=128))

            def dma_x(eng, b0, b1):
                sz = 128 * (b1 - b0) * 2 * 128
                o0 = x_off[(b0, b1)]
                eng.dma_start(
                    out=xp_sb[:, b0:b1],
                    in_=xp[o0:o0 + sz].rearrange("(p r) -> p r", p=128))

            dma_x(nc.sync, 0, 2)
            dma_w(nc.sync, W_GROUPS[2])       # q0 kc4-5
            dma_x(nc.sync, 2, 8)
            dma_w(nc.scalar, W_GROUPS[0])     # q0 kc0-1
            dma_w(nc.scalar, W_GROUPS[1])     # q0 kc2-3
            dma_w(nc.gpsimd, W_GROUPS[3])     # q0 kc6-7
            # Gate: a dummy copy that READS the kc6-7 slice (so it waits for
            # the last critical-set transfer) and WRITES into q1's landing
            # zone (a true WAW dep the scheduler cannot reorder around).
            # This keeps the 3.15 MB of q1-q3 weight bulk off the wire until
            # the stream-start bytes have landed; q2/q3 then serialize behind
            # q1 on the POOL queue's completion semaphores.
            nc.gpsimd.tensor_copy(wq_sb[:, 0, 0, 512:528],
                                  wq_sb[:, 6, 0, 0:16])
            dma_w(nc.gpsimd, W_GROUPS[4])     # q1
            dma_x(nc.gpsimd, 8, MT)
            dma_w(nc.gpsimd, W_GROUPS[5])     # q2
            dma_w(nc.gpsimd, W_GROUPS[6])     # q3

            # PE warmup: dummy matmuls on memset scratch keep the PE busy
            # through the HAM activity window while the first chunks land;
            # the first real matmuls still run at the cold 1.2 GHz clock but
            # retire work, and the stream is warm from ~13 us on.
            wu_ps = psum_pool.tile([128, 512], mybir.dt.float32,
                                   name="wu_ps", tag="ps")
            for _ in range(9):
                nc.tensor.matmul(wu_ps, wu_a, wu_b, start=True, stop=True,
                                 perf_mode=mybir.MatmulPerfMode.DoubleRow)

            xs_u32 = xs_sb.bitcast(mybir.dt.uint32)
            xp_u32 = xp_sb.bitcast(mybir.dt.uint32)

            def expand_x(m0, m1):
                # Sign bits -> fp8 {+1,-1}: bit kc shifted to each byte's MSB
                # (fp8 sign bit), then OR in 0x38 (the fp8e4 encoding of 1.0).
                # DVE instruction overhead is ~155 ns, so later groups batch
                # several mi per instruction; the first ones stay small to
                # unblock the matmul stream as early as possible.
                for kc in range(KC):
                    nc.vector.tensor_scalar(
                        out=xs_u32[:, m0:m1, kc], in0=xp_u32[:, m0:m1],
                        scalar1=7 - kc, scalar2=0x80808080,
                        op0=mybir.AluOpType.logical_shift_left,
                        op1=mybir.AluOpType.bitwise_and)
                nc.vector.tensor_scalar(
                    out=xs_u32[:, m0:m1], in0=xs_u32[:, m0:m1],
                    scalar1=0x38383838, scalar2=None,
                    op0=mybir.AluOpType.bitwise_or)

            # The whole expansion schedule is emitted up front: DVE has no
            # other work, so it runs the groups back-to-back, each gated only
            # by its xp DMA group.
            for m0, m1 in E_GROUPS:
                expand_x(m0, m1)

            # Dense fp8 DoubleRow matmuls: lhsT = xs[mi, kc] (stationary),
            # rhs = wq[kc, q-slice].  One PSUM bank per unit, 8-chunk
            # accumulation, then an f32 -> f16 ACT copy (exact) and a
            # contiguous 128 KB store per unit.
            for blk0 in range(0, MT, MI_BLK):
                for q in range(QT):
                    for mi in range(blk0, blk0 + MI_BLK):
                        ps = psum_pool.tile([128, 512], mybir.dt.float32,
                                            name="ps", tag="ps")
                        for kc in range(KC):
                            nc.tensor.matmul(
                                ps, xs_sb[:, mi, kc],
                                wq_sb[:, kc, :, q * 512:(q + 1) * 512],
                                start=(kc == 0), stop=(kc == KC - 1),
                                perf_mode=mybir.MatmulPerfMode.DoubleRow)
                        ot = out_pool.tile([128, 512], mybir.dt.float16,
                                           name="ot", tag="ot")
                        blk = mi * QT + q
                        if blk == MT * QT - 1:
                            # Final unit: split the copy across ACT+DVE and
                            # the store across two queues so the end-of-kernel
                            # copy->store->completion chain halves.
                            nc.scalar.copy(ot[0:64], ps[0:64])
                            nc.vector.tensor_copy(ot[64:128], ps[64:128])
                            nc.scalar.dma_start(
                                out=out[blk * 128:blk * 128 + 64],
                                in_=ot[0:64])
                            nc.sync.dma_start(
                                out=out[blk * 128 + 64:blk * 128 + 128],
                                in_=ot[64:128])
                        else:
                            nc.scalar.copy(ot, ps)
                            nc.sync.dma_start(
                                out=out[bass.ts(blk, 128)], in_=ot)

    nc.finalize()
    return nc


def ternarize_host(weight: np.ndarray) -> np.ndarray:
    """absmean ternarization, f64 for a faithful gamma; returns {-1,0,1} f32."""
    w = weight.astype(np.float64)
    gamma = np.mean(np.abs(w)) + EPS
    return (np.sign(w) * np.minimum(np.round(np.abs(w) / gamma), 1.0)).astype(
        np.float32)


def pack_w_flat(wq_t: np.ndarray) -> np.ndarray:
    """ternary Wq^T [i, o] f32 -> flat u8 (fp8e4 bytes), DMA-grouped."""
    # [kc, j, p, o] -> fp8 bytes
    w4 = wq_t.reshape(KC, 2, 128, O_DIM).astype(FP8).view(np.uint8)
    blocks = []
    for k0, k1, q in W_GROUPS:
        blk = w4[k0:k1, :, :, q * 512:(q + 1) * 512]     # [kcr, 2, 128, 512]
        blocks.append(np.ascontiguousarray(
            blk.transpose(2, 0, 1, 3)).reshape(-1))      # partition-major
    return np.concatenate(blocks)


def pack_x_flat(sh: np.ndarray) -> np.ndarray:
    """x shard [m_per, i] f32 -> flat u8 sign-bit planes, DMA-grouped.

    Byte (p, mi, j, m) holds bit kc = signbit(x[mi*128+m, kc*256+j*128+p]).
    """
    sb = np.signbit(sh)                                   # [m, i] bool
    # [kc, j, p, mi, m] -> [p, mi, j, m, kc]
    b = sb.T.reshape(KC, 2, 128, MT, 128).transpose(2, 3, 1, 4, 0)
    pk = np.packbits(np.ascontiguousarray(b), axis=-1,
                     bitorder="little")[..., 0]           # [128, MT, 2, 128]
    blocks = [np.ascontiguousarray(pk[:, b0:b1]).reshape(-1)
              for b0, b1 in X_GROUPS]
    return np.concatenate(blocks)


def prep_in_maps(x: np.ndarray, weight: np.ndarray) -> list[dict]:
    wq = ternarize_host(weight)                    # [o, i] ternary
    wt = pack_w_flat(np.ascontiguousarray(wq.T))
    xf = x.reshape(M_TOT, I_DIM)
    return [{"xp": pack_x_flat(xf[c * M_PER:(c + 1) * M_PER]), "wt": wt}
            for c in range(N_CORES)]


_PROGRAM_CACHE: dict = {}


def _get_program() -> bass.Bass:
    if "nc" not in _PROGRAM_CACHE:
        _PROGRAM_CACHE["nc"] = build_program()
    return _PROGRAM_CACHE["nc"]


def _gather(results: list[dict]) -> np.ndarray:
    # per-core out [MT*QT*128, 512] -> [m_per, o]
    shards = [
        np.asarray(r["out"]).reshape(MT, QT, 128, 512)
        .transpose(0, 2, 1, 3).reshape(M_PER, O_DIM)
        for r in results]
    full = np.concatenate(shards, axis=0)
    return np.ascontiguousarray(full.reshape(B, S, O_DIM).astype(np.float32))


def kernel(x: np.ndarray, weight: np.ndarray) -> np.ndarray:
    nc = _get_program()
    in_maps = prep_in_maps(np.asarray(x), np.asarray(weight))
    res = run_bass_kernel_spmd(nc, in_maps, core_ids=list(range(N_CORES)))
    return _gather(res.results)


def kernel_traced(x: np.ndarray, weight: np.ndarray, **trace_kw):
    """Like kernel() but returns (output, BassKernelResults) with a trace."""
    nc = _get_program()
    in_maps = prep_in_maps(np.asarray(x), np.asarray(weight))
    res = run_bass_kernel_spmd(
        nc, in_maps, core_ids=list(range(N_CORES)), trace=True, **trace_kw)
    return _gather(res.results), res
